# revision 9
# baseline (speedup 1.0000x reference)
"""Trainium2 Bass kernel for the dense MoE layer (nn_MoELayer_74371653698164).

Reference computation (fp32):
    gate  = softmax(x @ Wg + bg)                    # [N, E]
    out   = sum_e gate[:, e] * (x @ We[e] + be[e])  # [N, D_OUT]

v2 strategy (hardware loop):
  - Data-parallel over tokens: each of the 8 cores gets N/8 = 1024 tokens and
    the full expert/gate weights. No collectives.
  - The ENTIRE per-invocation computation (input DMAs from HBM, gate, expert
    matmuls, gate-weighted accumulation, output store) is the body of a
    tc.For_i hardware loop. In this environment the runtime pays a large
    per-STATIC-instruction dispatch cost on every execution, while re-running
    the same instructions via the loop back-edge executes at true silicon
    speed. reps = loop trip count; NEFFs for different reps are structurally
    identical, so (wall[R] - wall[1]) / (R-1) isolates the real per-rep
    hardware time.
  - Same math as v1: softmax factored as r * sum_e exp_e (x @ We[e] + be[e]),
    logits computed transposed ([E, tok]) so bias-add is per-partition and
    exp is a single activation; bias term seeds acc via exp @ be matmuls;
    per (expert, token-tile) one DVE FMA folds the gate weight in.
  - Matmul operands are float32r (fp32 data, 1 cycle/row PE streaming).
  - All SBUF/PSUM tiles are allocated once, outside the loop; double
    buffering (We DMA, PSUM pairs) is explicit.

kernel(**inputs) takes the FULL unsharded inputs and returns the FULL output.
"""
import os
from contextlib import ExitStack

import numpy as np

import bass_rust
import concourse.bass as bass
import concourse.mybir as mybir
import concourse.tile as tile
from concourse.bass_utils import run_bass_kernel_spmd
from concourse.masks import make_identity
from concourse.vector_clock import ScopedClock

# Problem shape (hardcoded per harness contract).
N_TOKENS, D_IN, D_OUT, E = 8192, 1024, 1024, 8
NCORES = 8
TOK = N_TOKENS // NCORES  # tokens per core
P = 128                   # partitions
KT = D_IN // P            # contraction tiles
TT = TOK // P             # token tiles per core
FH = 512                  # max fp32 matmul free dim (one PSUM bank)

MODE = os.environ.get("MOE_KERNEL_MODE", "f32r")

_F32 = mybir.dt.float32
_F32R = mybir.dt.float32r
_BF16 = mybir.dt.bfloat16


class _ChunkedDrainTileContext(tile.TileContext):
    """TileContext adapted to a walrus that allows ONE sync wait per
    instruction (see v1 docstring)."""

    _HOIST_WAITS = os.environ.get("MOE_HOIST_WAITS", "0") == "1"

    def __init__(self, *a, **kw):
        super().__init__(*a, **kw)
        self._last_by_engine = {}

    def _add_instruction(self, inst):
        si = getattr(inst, "sync_info", None)
        if si is not None and si.on_wait and len(si.on_wait) > 1:
            waits = list(si.on_wait)
            if self._HOIST_WAITS and len(waits) == 2:
                prev = self._last_by_engine.get(inst.engine)
                psi = getattr(prev, "sync_info", None) if prev is not None else None
                if prev is not None and (
                    psi is None or (not psi.on_wait and not psi.on_update)
                ):
                    prev.sync_info = bass_rust.SyncInfo(
                        on_wait=[waits[0]], on_update=[])
                    waits = waits[1:]
            for w in waits[:-1]:
                nop = mybir.InstNoOp(
                    name=self.nc.get_next_instruction_name(), ins=[], outs=[]
                )
                nop.engine = inst.engine
                nop.bass_nofuse = True
                nop.sync_info = bass_rust.SyncInfo(on_wait=[w], on_update=[])
                super()._add_instruction(nop)
            inst.sync_info = bass_rust.SyncInfo(
                on_wait=[waits[-1]], on_update=list(si.on_update or [])
            )
        self._last_by_engine[inst.engine] = inst
        super()._add_instruction(inst)

    def _drain_and_barrier(self, tick_clock, wait_clock):
        drain_inst = self.nc.sync.drain()
        wait_clock.add_sem_waits(
            drain_inst.ins, ScopedClock({None: tick_clock.global_clock})
        )
        si = drain_inst.ins.sync_info
        waits = list(si.on_wait or []) if si is not None else []
        if len(waits) > 1:
            drain_inst.ins.sync_info = bass_rust.SyncInfo(
                on_wait=waits[:1], on_update=list(si.on_update or [])
            )
            for w in waits[1:]:
                extra = self.nc.sync.drain()
                extra.ins.sync_info = bass_rust.SyncInfo(on_wait=[w], on_update=[])

        self.nc.all_engine_barrier()
        assert self.sems is not None
        popped = self.nc._tile_sem_poison_stack.pop()
        assert popped is self._sem_poison
        self.nc.clear_and_free_semaphores(list(self.sems.allocated().values()))
        self.nc.all_engine_barrier()


def build_nc(mode: str = MODE, reps: int = 1, internal_io: bool = False) -> bass.Bass:
    """Build the per-core Bass program. reps = For_i trip count."""
    mmdt = {"bf16": _BF16, "f32": _F32, "f32r": _F32R}[mode]

    nc = bass.Bass()
    kind_in = {} if internal_io else {"kind": "ExternalInput"}
    xT_d = nc.dram_tensor("xT", [D_IN, TOK], mmdt, **kind_in)
    We_d = nc.dram_tensor("We", [E, D_IN, D_OUT], mmdt, **kind_in)
    be_d = nc.dram_tensor("be", [E, D_OUT], mmdt, **kind_in)
    Wg_d = nc.dram_tensor("Wg", [D_IN, E], mmdt, **kind_in)
    bg_d = nc.dram_tensor("bg", [E], _F32, **kind_in)
    if internal_io:
        out_d = nc.dram_tensor("out", [TOK, D_OUT], _F32)
        probe_d = nc.dram_tensor("probe", [P, P], _F32, kind="ExternalOutput")
        cnt_d = nc.dram_tensor("cnt", [P, 1], _F32, kind="ExternalOutput")
    else:
        out_d = nc.dram_tensor("out", [TOK, D_OUT], _F32, kind="ExternalOutput")
        probe_d = None
        cnt_d = None

    with _ChunkedDrainTileContext(nc) as tc, ExitStack() as ctx:
        singles = ctx.enter_context(tc.tile_pool(name="singles", bufs=1))
        pspool = ctx.enter_context(tc.tile_pool(name="ps", bufs=1, space="PSUM"))

        if internal_io:
            seed = singles.tile([P, D_OUT], _F32, tag="seed")
            nc.vector.memset(seed[:], 0.005)

            def as_dt(ap, dtt):
                return ap if ap.dtype == dtt else ap.bitcast(dtt)

            def rep_src(n_rep):
                s = seed[:, :].opt()
                return bass.AP(tensor=s.tensor, offset=s.offset,
                               ap=[[s.ap[0][0], P], [0, n_rep], [1, D_OUT]])

            nc.sync.dma_start(xT_d.rearrange("(k p) n -> p k n", p=P),
                              as_dt(rep_src(KT), mmdt))
            for e in range(E):
                nc.sync.dma_start(We_d[e].rearrange("(k p) o -> p k o", p=P),
                                  as_dt(rep_src(KT), mmdt))
            nc.sync.dma_start(be_d[:, :], as_dt(seed[0:E, :], mmdt))
            nc.sync.dma_start(Wg_d.rearrange("(k p) e -> p k e", p=P),
                              as_dt(seed[:, 0:KT * E].rearrange(
                                  "p (k e) -> p k e", k=KT), mmdt))
            nc.sync.dma_start(bg_d[:], seed[0, 0:E])

        # constants (not input data): identities for the tiny transposes and
        # an all-ones stationary for the cross-expert sum broadcast
        ident8f = singles.tile([E, E], _F32, tag="id8f")
        make_identity(nc, ident8f)
        if mmdt != _F32:
            ident8 = singles.tile([E, E], mmdt, tag="id8")
            nc.scalar.copy(ident8[:], ident8f[:])
        else:
            ident8 = ident8f
        ones8f = singles.tile([E, E], _F32, tag="ones8f")
        nc.vector.memset(ones8f[:], 1.0)
        if mmdt != _F32:
            ones8 = singles.tile([E, E], mmdt, tag="ones8")
            nc.scalar.copy(ones8[:], ones8f[:])
        else:
            ones8 = ones8f

        # ---- all tiles allocated once, outside the hardware loop ----
        wg_sb = singles.tile([P, KT, E], mmdt, tag="wg")
        bg_col = singles.tile([E, 1], _F32, tag="bg")
        be_sb = singles.tile([E, D_OUT], mmdt, tag="be")
        xT = singles.tile([P, KT, TOK], mmdt, tag="xT")
        acc = singles.tile([P, TT, D_OUT], _F32, tag="acc")
        expT = singles.tile([E, TOK], mmdt, tag="expT")
        rsum = singles.tile([E, TOK], _F32, tag="rsum")
        expTn = singles.tile([E, TOK], mmdt, tag="expTn")
        exp_tok = singles.tile([P, TT, E], _F32, tag="exptok")
        we_bufs = [singles.tile([P, KT, D_OUT], mmdt, tag=f"we{b}",
                                name=f"we{b}") for b in range(2)]
        pm = [pspool.tile([P, D_OUT], _F32, tag=f"pm{b}", name=f"pm{b}")
              for b in range(2)]
        pg = pspool.tile([E, TOK], _F32, tag="pg")
        ptr = pspool.tile([P, TT * E], mmdt, tag="ptr")

        if internal_io:
            # loop-iteration counter so the timing harness can verify the
            # trip count actually executed
            cnt = singles.tile([P, 1], _F32, tag="cnt")
            one = singles.tile([P, 1], _F32, tag="one")
            nc.vector.memset(cnt[:], 0.0)
            nc.vector.memset(one[:], 1.0)

        with tc.For_i(0, reps):
            if internal_io:
                nc.vector.tensor_scalar_add(cnt[:], cnt[:], one[:])
            # full input load from HBM every iteration; smalls first, then
            # xT in halves (lets gate matmuls start early), then the We
            # stream (double-buffered against the expert matmuls)
            nc.sync.dma_start(wg_sb[:], Wg_d.rearrange("(k p) e -> p k e", p=P))
            nc.sync.dma_start(bg_col[:], bg_d[:])
            nc.sync.dma_start(be_sb[:], be_d[:, :])
            xT_src = xT_d.rearrange("(k p) n -> p k n", p=P)
            nc.sync.dma_start(xT[:, 0:KT // 2, :], xT_src[:, 0:KT // 2, :])
            nc.sync.dma_start(xT[:, KT // 2:KT, :], xT_src[:, KT // 2:KT, :])

            # gate: logits^T [E, tok] in PSUM, bias add, exp; then the
            # softmax denominator via an all-ones matmul (row-broadcast sum),
            # reciprocal, and a single scale -> normalized gate weights expTn
            for k in range(KT):
                for h in range(TOK // FH):
                    nc.tensor.matmul(
                        pg[:, h * FH:(h + 1) * FH], wg_sb[:, k, :],
                        xT[:, k, h * FH:(h + 1) * FH],
                        start=(k == 0), stop=(k == KT - 1),
                    )
            # fused: expT = exp(logitsT + bg), ACT reading PSUM directly
            nc.scalar.activation(expT[:], pg[:],
                                 mybir.ActivationFunctionType.Exp,
                                 bias=bg_col[:])
            for h in range(TOK // FH):
                nc.tensor.matmul(pg[:, h * FH:(h + 1) * FH], ones8[:],
                                 expT[:, h * FH:(h + 1) * FH],
                                 start=True, stop=True)
            nc.vector.reciprocal(rsum[:], pg[:])
            nc.vector.tensor_mul(expTn[:], expT[:], rsum[:])
            for i in range(TT):
                nc.tensor.transpose(ptr[:, i * E:(i + 1) * E],
                                    expTn[:, i * P:(i + 1) * P], ident8[:])
            nc.scalar.copy(exp_tok.rearrange("p a b -> p (a b)"), ptr[:])

            # acc init: normalized-gate @ be
            for i in range(TT):
                pb = pm[i % 2]
                for h in range(D_OUT // FH):
                    nc.tensor.matmul(
                        pb[:, h * FH:(h + 1) * FH],
                        expTn[:, i * P:(i + 1) * P],
                        be_sb[:, h * FH:(h + 1) * FH],
                        start=True, stop=True,
                    )
                nc.scalar.copy(acc[:, i, :], pb[:])

            # experts: acc += gate[:, e] * (x @ We[e]); after the last
            # expert's FMA each token tile is final -> store it immediately
            out_dst = out_d.rearrange("(i p) o -> p i o", p=P)
            for e in range(E):
                we = we_bufs[e % 2]
                nc.sync.dma_start(
                    we[:], We_d[e].rearrange("(k p) o -> p k o", p=P))
                for i in range(TT):
                    isl = slice(i * P, (i + 1) * P)
                    pmt = pm[i % 2]
                    for k in range(KT):
                        for h in range(D_OUT // FH):
                            nc.tensor.matmul(
                                pmt[:, h * FH:(h + 1) * FH], xT[:, k, isl],
                                we[:, k, h * FH:(h + 1) * FH],
                                start=(k == 0), stop=(k == KT - 1),
                            )
                    nc.vector.scalar_tensor_tensor(
                        out=acc[:, i, :], in0=pmt[:],
                        scalar=exp_tok[:, i, e:e + 1], in1=acc[:, i, :],
                        op0=mybir.AluOpType.mult, op1=mybir.AluOpType.add,
                    )
                    if e == E - 1:
                        nc.sync.dma_start(out_dst[:, i, :], acc[:, i, :])

        if internal_io:
            nc.sync.dma_start(probe_d[:, :], acc[:, 0, 0:P])
            nc.sync.dma_start(cnt_d[:, :], cnt[:])

    return nc


_NC_CACHE: dict = {}


def _get_nc(mode: str, reps: int = 1) -> bass.Bass:
    key = (mode, reps)
    if key not in _NC_CACHE:
        _NC_CACHE[key] = build_nc(mode, reps)
    return _NC_CACHE[key]


def make_in_maps(x, We, be, Wg, bg, mode: str = MODE):
    import ml_dtypes

    dt_np = ml_dtypes.bfloat16 if mode == "bf16" else np.float32
    We_c = np.ascontiguousarray(We, dtype=dt_np)
    be_c = np.ascontiguousarray(be, dtype=dt_np)
    Wg_c = np.ascontiguousarray(Wg, dtype=dt_np)
    bg_c = np.ascontiguousarray(bg, dtype=np.float32)
    in_maps = []
    for c in range(NCORES):
        xs = np.asarray(x[c * TOK:(c + 1) * TOK], dtype=dt_np)
        in_maps.append({
            "xT": np.ascontiguousarray(xs.T),
            "We": We_c,
            "be": be_c,
            "Wg": Wg_c,
            "bg": bg_c,
        })
    return in_maps


def kernel(x, We, be, Wg, bg):
    nc = _get_nc(MODE)
    in_maps = make_in_maps(x, We, be, Wg, bg, MODE)
    res = run_bass_kernel_spmd(nc, in_maps, list(range(NCORES)))
    out = np.concatenate([res.results[c]["out"] for c in range(NCORES)], axis=0)
    return out.astype(np.float32)


# revision 11
# speedup vs baseline: 1.0376x; 1.0376x over previous
"""Trainium2 Bass kernel for the dense MoE layer (nn_MoELayer_74371653698164).

Reference computation (fp32):
    gate  = softmax(x @ Wg + bg)                    # [N, E]
    out   = sum_e gate[:, e] * (x @ We[e] + be[e])  # [N, D_OUT]

v2 strategy (hardware loop):
  - Data-parallel over tokens: each of the 8 cores gets N/8 = 1024 tokens and
    the full expert/gate weights. No collectives.
  - The ENTIRE per-invocation computation (input DMAs from HBM, gate, expert
    matmuls, gate-weighted accumulation, output store) is the body of a
    tc.For_i hardware loop. In this environment the runtime pays a large
    per-STATIC-instruction dispatch cost on every execution, while re-running
    the same instructions via the loop back-edge executes at true silicon
    speed. reps = loop trip count; NEFFs for different reps are structurally
    identical, so (wall[R] - wall[1]) / (R-1) isolates the real per-rep
    hardware time.
  - Same math as v1: softmax factored as r * sum_e exp_e (x @ We[e] + be[e]),
    logits computed transposed ([E, tok]) so bias-add is per-partition and
    exp is a single activation; bias term seeds acc via exp @ be matmuls;
    per (expert, token-tile) one DVE FMA folds the gate weight in.
  - Matmul operands are float32r (fp32 data, 1 cycle/row PE streaming).
  - All SBUF/PSUM tiles are allocated once, outside the loop; double
    buffering (We DMA, PSUM pairs) is explicit.

kernel(**inputs) takes the FULL unsharded inputs and returns the FULL output.
"""
import os
from contextlib import ExitStack

import numpy as np

import bass_rust
import concourse.bass as bass
import concourse.mybir as mybir
import concourse.tile as tile
from concourse.bass_utils import run_bass_kernel_spmd
from concourse.masks import make_identity
from concourse.vector_clock import ScopedClock

# Problem shape (hardcoded per harness contract).
N_TOKENS, D_IN, D_OUT, E = 8192, 1024, 1024, 8
NCORES = 8
TOK = N_TOKENS // NCORES  # tokens per core
P = 128                   # partitions
KT = D_IN // P            # contraction tiles
TT = TOK // P             # token tiles per core
FH = 512                  # max fp32 matmul free dim (one PSUM bank)

MODE = os.environ.get("MOE_KERNEL_MODE", "f32r")

_F32 = mybir.dt.float32
_F32R = mybir.dt.float32r
_BF16 = mybir.dt.bfloat16


class _ChunkedDrainTileContext(tile.TileContext):
    """TileContext adapted to a walrus that allows ONE sync wait per
    instruction (see v1 docstring)."""

    _HOIST_WAITS = os.environ.get("MOE_HOIST_WAITS", "0") == "1"

    def __init__(self, *a, **kw):
        super().__init__(*a, **kw)
        self._last_by_engine = {}

    def _add_instruction(self, inst):
        si = getattr(inst, "sync_info", None)
        if si is not None and si.on_wait and len(si.on_wait) > 1:
            waits = list(si.on_wait)
            if self._HOIST_WAITS and len(waits) == 2:
                prev = self._last_by_engine.get(inst.engine)
                psi = getattr(prev, "sync_info", None) if prev is not None else None
                if prev is not None and (
                    psi is None or (not psi.on_wait and not psi.on_update)
                ):
                    prev.sync_info = bass_rust.SyncInfo(
                        on_wait=[waits[0]], on_update=[])
                    waits = waits[1:]
            for w in waits[:-1]:
                nop = mybir.InstNoOp(
                    name=self.nc.get_next_instruction_name(), ins=[], outs=[]
                )
                nop.engine = inst.engine
                nop.bass_nofuse = True
                nop.sync_info = bass_rust.SyncInfo(on_wait=[w], on_update=[])
                super()._add_instruction(nop)
            inst.sync_info = bass_rust.SyncInfo(
                on_wait=[waits[-1]], on_update=list(si.on_update or [])
            )
        self._last_by_engine[inst.engine] = inst
        super()._add_instruction(inst)

    def _drain_and_barrier(self, tick_clock, wait_clock):
        drain_inst = self.nc.sync.drain()
        wait_clock.add_sem_waits(
            drain_inst.ins, ScopedClock({None: tick_clock.global_clock})
        )
        si = drain_inst.ins.sync_info
        waits = list(si.on_wait or []) if si is not None else []
        if len(waits) > 1:
            drain_inst.ins.sync_info = bass_rust.SyncInfo(
                on_wait=waits[:1], on_update=list(si.on_update or [])
            )
            for w in waits[1:]:
                extra = self.nc.sync.drain()
                extra.ins.sync_info = bass_rust.SyncInfo(on_wait=[w], on_update=[])

        self.nc.all_engine_barrier()
        assert self.sems is not None
        popped = self.nc._tile_sem_poison_stack.pop()
        assert popped is self._sem_poison
        self.nc.clear_and_free_semaphores(list(self.sems.allocated().values()))
        self.nc.all_engine_barrier()


def build_nc(mode: str = MODE, reps: int = 1, internal_io: bool = False) -> bass.Bass:
    """Build the per-core Bass program. reps = For_i trip count."""
    mmdt = {"bf16": _BF16, "f32": _F32, "f32r": _F32R}[mode]

    nc = bass.Bass()
    kind_in = {} if internal_io else {"kind": "ExternalInput"}
    xT_d = nc.dram_tensor("xT", [D_IN, TOK], mmdt, **kind_in)
    We_d = nc.dram_tensor("We", [E, D_IN, D_OUT], mmdt, **kind_in)
    be_d = nc.dram_tensor("be", [E, D_OUT], mmdt, **kind_in)
    Wg_d = nc.dram_tensor("Wg", [D_IN, E], mmdt, **kind_in)
    bg_d = nc.dram_tensor("bg", [E], _F32, **kind_in)
    if internal_io:
        out_d = nc.dram_tensor("out", [TOK, D_OUT], _F32)
        probe_d = nc.dram_tensor("probe", [P, P], _F32, kind="ExternalOutput")
        cnt_d = nc.dram_tensor("cnt", [P, 1], _F32, kind="ExternalOutput")
    else:
        out_d = nc.dram_tensor("out", [TOK, D_OUT], _F32, kind="ExternalOutput")
        probe_d = None
        cnt_d = None

    with _ChunkedDrainTileContext(nc) as tc, ExitStack() as ctx:
        singles = ctx.enter_context(tc.tile_pool(name="singles", bufs=1))
        pspool = ctx.enter_context(tc.tile_pool(name="ps", bufs=1, space="PSUM"))

        if internal_io:
            seed = singles.tile([P, D_OUT], _F32, tag="seed")
            nc.vector.memset(seed[:], 0.005)

            def as_dt(ap, dtt):
                return ap if ap.dtype == dtt else ap.bitcast(dtt)

            def rep_src(n_rep):
                s = seed[:, :].opt()
                return bass.AP(tensor=s.tensor, offset=s.offset,
                               ap=[[s.ap[0][0], P], [0, n_rep], [1, D_OUT]])

            nc.sync.dma_start(xT_d.rearrange("(k p) n -> p k n", p=P),
                              as_dt(rep_src(KT), mmdt))
            for e in range(E):
                nc.sync.dma_start(We_d[e].rearrange("(k p) o -> p k o", p=P),
                                  as_dt(rep_src(KT), mmdt))
            nc.sync.dma_start(be_d[:, :], as_dt(seed[0:E, :], mmdt))
            nc.sync.dma_start(Wg_d.rearrange("(k p) e -> p k e", p=P),
                              as_dt(seed[:, 0:KT * E].rearrange(
                                  "p (k e) -> p k e", k=KT), mmdt))
            nc.sync.dma_start(bg_d[:], seed[0, 0:E])

        # constants (not input data): identities for the tiny transposes and
        # an all-ones stationary for the cross-expert sum broadcast
        ident8f = singles.tile([E, E], _F32, tag="id8f")
        make_identity(nc, ident8f)
        if mmdt != _F32:
            ident8 = singles.tile([E, E], mmdt, tag="id8")
            nc.scalar.copy(ident8[:], ident8f[:])
        else:
            ident8 = ident8f
        ones8f = singles.tile([E, E], _F32, tag="ones8f")
        nc.vector.memset(ones8f[:], 1.0)
        if mmdt != _F32:
            ones8 = singles.tile([E, E], mmdt, tag="ones8")
            nc.scalar.copy(ones8[:], ones8f[:])
        else:
            ones8 = ones8f

        # ---- all tiles allocated once, outside the hardware loop ----
        wg_sb = singles.tile([P, KT, E], mmdt, tag="wg")
        bg_col = singles.tile([E, 1], _F32, tag="bg")
        be_sb = singles.tile([E, D_OUT], mmdt, tag="be")
        xT = singles.tile([P, KT, TOK], mmdt, tag="xT")
        acc = singles.tile([P, TT, D_OUT], _F32, tag="acc")
        ltT = singles.tile([E, TOK], _F32, tag="ltT")
        expT = singles.tile([E, TOK], mmdt, tag="expT")
        rsum = singles.tile([E, TOK], _F32, tag="rsum")
        expTn = singles.tile([E, TOK], mmdt, tag="expTn")
        exp_tok = singles.tile([P, TT, E], _F32, tag="exptok")
        we_bufs = [singles.tile([P, KT, D_OUT], mmdt, tag=f"we{b}",
                                name=f"we{b}") for b in range(2)]
        pm = [pspool.tile([P, D_OUT], _F32, tag=f"pm{b}", name=f"pm{b}")
              for b in range(2)]
        pg = pspool.tile([E, TOK], _F32, tag="pg")
        ptr = pspool.tile([P, TT * E], mmdt, tag="ptr")

        if internal_io:
            # loop-iteration counter so the timing harness can verify the
            # trip count actually executed
            cnt = singles.tile([P, 1], _F32, tag="cnt")
            one = singles.tile([P, 1], _F32, tag="one")
            nc.vector.memset(cnt[:], 0.0)
            nc.vector.memset(one[:], 1.0)

        with tc.For_i(0, reps):
            if internal_io:
                nc.vector.tensor_scalar_add(cnt[:], cnt[:], one[:])
            # full input load from HBM every iteration; smalls first, then
            # xT in halves (lets gate matmuls start early), then the We
            # stream (double-buffered against the expert matmuls)
            nc.sync.dma_start(wg_sb[:], Wg_d.rearrange("(k p) e -> p k e", p=P))
            nc.sync.dma_start(bg_col[:], bg_d[:])
            nc.sync.dma_start(be_sb[:], be_d[:, :])
            xT_src = xT_d.rearrange("(k p) n -> p k n", p=P)
            nc.sync.dma_start(xT[:, 0:KT // 2, :], xT_src[:, 0:KT // 2, :])
            nc.sync.dma_start(xT[:, KT // 2:KT, :], xT_src[:, KT // 2:KT, :])

            # gate: logits^T [E, tok] in PSUM, bias add, exp; then the
            # softmax denominator via an all-ones matmul (row-broadcast sum),
            # reciprocal, and a single scale -> normalized gate weights expTn
            for k in range(KT):
                for h in range(TOK // FH):
                    nc.tensor.matmul(
                        pg[:, h * FH:(h + 1) * FH], wg_sb[:, k, :],
                        xT[:, k, h * FH:(h + 1) * FH],
                        start=(k == 0), stop=(k == KT - 1),
                    )
            nc.vector.tensor_scalar_add(ltT[:], pg[:], bg_col[:])
            nc.scalar.activation(expT[:], ltT[:],
                                 mybir.ActivationFunctionType.Exp)
            for h in range(TOK // FH):
                nc.tensor.matmul(pg[:, h * FH:(h + 1) * FH], ones8[:],
                                 expT[:, h * FH:(h + 1) * FH],
                                 start=True, stop=True)
            nc.vector.reciprocal(rsum[:], pg[:])
            nc.vector.tensor_mul(expTn[:], expT[:], rsum[:])
            for i in range(TT):
                nc.tensor.transpose(ptr[:, i * E:(i + 1) * E],
                                    expTn[:, i * P:(i + 1) * P], ident8[:])
            nc.scalar.copy(exp_tok.rearrange("p a b -> p (a b)"), ptr[:])

            # acc init: normalized-gate @ be
            for i in range(TT):
                pb = pm[i % 2]
                for h in range(D_OUT // FH):
                    nc.tensor.matmul(
                        pb[:, h * FH:(h + 1) * FH],
                        expTn[:, i * P:(i + 1) * P],
                        be_sb[:, h * FH:(h + 1) * FH],
                        start=True, stop=True,
                    )
                nc.scalar.copy(acc[:, i, :], pb[:])

            # experts: acc += gate[:, e] * (x @ We[e]); after the last
            # expert's FMA each token tile is final -> store it immediately
            out_dst = out_d.rearrange("(i p) o -> p i o", p=P)
            for e in range(E):
                we = we_bufs[e % 2]
                nc.sync.dma_start(
                    we[:], We_d[e].rearrange("(k p) o -> p k o", p=P))
                for i in range(TT):
                    isl = slice(i * P, (i + 1) * P)
                    pmt = pm[i % 2]
                    for k in range(KT):
                        for h in range(D_OUT // FH):
                            nc.tensor.matmul(
                                pmt[:, h * FH:(h + 1) * FH], xT[:, k, isl],
                                we[:, k, h * FH:(h + 1) * FH],
                                start=(k == 0), stop=(k == KT - 1),
                            )
                    nc.vector.scalar_tensor_tensor(
                        out=acc[:, i, :], in0=pmt[:],
                        scalar=exp_tok[:, i, e:e + 1], in1=acc[:, i, :],
                        op0=mybir.AluOpType.mult, op1=mybir.AluOpType.add,
                    )
                    if e == E - 1:
                        nc.sync.dma_start(out_dst[:, i, :], acc[:, i, :])

        if internal_io:
            nc.sync.dma_start(probe_d[:, :], acc[:, 0, 0:P])
            nc.sync.dma_start(cnt_d[:, :], cnt[:])

    return nc


_NC_CACHE: dict = {}


def _get_nc(mode: str, reps: int = 1) -> bass.Bass:
    key = (mode, reps)
    if key not in _NC_CACHE:
        _NC_CACHE[key] = build_nc(mode, reps)
    return _NC_CACHE[key]


def make_in_maps(x, We, be, Wg, bg, mode: str = MODE):
    import ml_dtypes

    dt_np = ml_dtypes.bfloat16 if mode == "bf16" else np.float32
    We_c = np.ascontiguousarray(We, dtype=dt_np)
    be_c = np.ascontiguousarray(be, dtype=dt_np)
    Wg_c = np.ascontiguousarray(Wg, dtype=dt_np)
    bg_c = np.ascontiguousarray(bg, dtype=np.float32)
    in_maps = []
    for c in range(NCORES):
        xs = np.asarray(x[c * TOK:(c + 1) * TOK], dtype=dt_np)
        in_maps.append({
            "xT": np.ascontiguousarray(xs.T),
            "We": We_c,
            "be": be_c,
            "Wg": Wg_c,
            "bg": bg_c,
        })
    return in_maps


def kernel(x, We, be, Wg, bg):
    nc = _get_nc(MODE)
    in_maps = make_in_maps(x, We, be, Wg, bg, MODE)
    res = run_bass_kernel_spmd(nc, in_maps, list(range(NCORES)))
    out = np.concatenate([res.results[c]["out"] for c in range(NCORES)], axis=0)
    return out.astype(np.float32)
